# revision 10
# baseline (speedup 1.0000x reference)
"""MoE routing kernel for Trainium2 (8 NeuronCores, I-split parallel).

Problem: nn_MoDE_52140902973544 (moe_routing).
  x[4,2048,1024], router (8 experts, top-2, capacity 1024), 7 real experts
  with FFN H=1024 -> I=4096 -> H=1024 (relu), expert 7 = identity (noop).

Strategy (uniform load balance across all 8 cores):
  * Host: router forward + top-2 + capacity-limited dispatch (pure index
    math, order-based -> float-robust); gather dispatched tokens for all
    7 experts transposed into one [H, 7*cap] tensor (shared by cores).
  * Device (SPMD over 8 cores): core c owns I-rows [c*512,(c+1)*512) of
    every expert:  for each expert e:
        h_c  = relu(Wi_e[:, c-slice].T @ X_e)        # [512, cap]
        outp = Wo_e[c-slice, :].T @ h_c              # partial over I
    and writes bf16 partials outT_c [H, 7*cap].  Every core does
    896 matmuls (7/8 of the single-expert variant) -> uniform.
  * Host: sum the 8 partials (fp32), then combine via pure gathers
    (no scatter) + gate weights + noop path.
"""

import os
import sys

for _p in ("/opt/trn_rl_repo", "/opt/pypackages"):
    if _p not in sys.path:
        sys.path.append(_p)

import numpy as np

# ---- problem constants (hardcoded per contract) ----
B, S, H, I = 4, 2048, 1024, 4096
E = 8                 # experts incl. noop (last)
ER = E - 1            # real experts
TOP_K = 2
N_TOK = B * S         # 8192
CAP = 1024            # ceil(N_TOK / E * 1.0)
N_CORES = 8

P = 128               # partitions
KO = H // P           # 8   H chunks
ISL = I // N_CORES    # 512 I-slice per core
ICS = ISL // P        # 4   I chunks per core slice
NF = 512              # matmul free dim
NN = CAP // NF        # 2   cap tiles

MM_DTYPE = os.environ.get("MOE_MM_DTYPE", "bf16")

_CACHE = {}


def _build_nc(mm_dtype: str, repeat: int = 1):
    """Single-core Bass program (SPMD; identical across the 8 cores, the
    per-core I-slice comes in via the wi/wo input tensors).

    Per repeat (= one full forward):  for each of the 7 experts, DMA the
    expert's dispatched tokens [H, cap] + weight slices, GEMM1 into a
    4-chunk h tile, relu, GEMM2 (contraction over the 512 I-slice) into
    bf16 partial outT.  Weight+x DMA rides the SP HW-DGE queue, the
    outbound partial rides the Activation queue; pools are ring-buffered
    so DMA overlaps the PE stream.
    """
    import concourse.bacc as bacc
    import concourse.mybir as mybir
    import concourse.tile as tile

    dt = mybir.dt
    assert mm_dtype == "bf16"
    DT = dt.bfloat16

    nc = bacc.Bacc("TRN2")
    # x: all experts' capacity slots, token-major columns  [H, ER*CAP]
    xT = nc.declare_dram_parameter("xT", [H, ER * CAP], DT, isOutput=False)
    # wi: per-core I-slice of every expert  [ER*H, ISL]
    wi = nc.declare_dram_parameter("wi", [ER * H, ISL], DT, isOutput=False)
    # wo: per-core I-slice of every expert  [ER*ISL, H]
    wo = nc.declare_dram_parameter("wo", [ER * ISL, H], DT, isOutput=False)
    # bf16 partial output (summed on host across cores)
    outT = nc.declare_dram_parameter("outT", [H, ER * CAP], DT, isOutput=True)

    with tile.TileContext(nc) as tc:
        from contextlib import ExitStack

        with ExitStack() as ctx:
            xpool = ctx.enter_context(tc.tile_pool(name="x", bufs=3))
            wipool = ctx.enter_context(tc.tile_pool(name="wi", bufs=3))
            wopool = ctx.enter_context(tc.tile_pool(name="wo", bufs=3))
            hpool = ctx.enter_context(tc.tile_pool(name="h", bufs=2))
            opool = ctx.enter_context(tc.tile_pool(name="o", bufs=3))
            ps1pool = ctx.enter_context(
                tc.tile_pool(name="ps1", bufs=4, space="PSUM"))
            ps2pool = ctx.enter_context(
                tc.tile_pool(name="ps2", bufs=4, space="PSUM"))

            # [128, ko, e*cap] / [128, e*ko, isl] / [128, e*ics, h] views
            x_r = xT.rearrange("(ko p) n -> p ko n", p=P)
            wi_r = wi.rearrange("(e ko p) i -> p e ko i", p=P, e=ER)
            wo_r = wo.rearrange("(e ki p) h -> p e ki h", p=P, e=ER)
            o_r = outT.rearrange("(ko p) n -> p ko n", p=P)

          # fmt: off
          # noqa
            for _rep in range(repeat):
              for e in range(ER):
                  x_sb = xpool.tile([P, KO, CAP], DT, tag="x", name="x_sb")
                  nc.sync.dma_start(
                      x_sb[:], x_r[:, :, e * CAP:(e + 1) * CAP])
                  wi_sb = wipool.tile([P, KO, ISL], DT, tag="wi", name="wi_sb")
                  nc.sync.dma_start(wi_sb[:], wi_r[:, e])
                  wo_sb = wopool.tile([P, ICS, H], DT, tag="wo", name="wo_sb")
                  nc.scalar.dma_start(wo_sb[:], wo_r[:, e])

                  # ---- GEMM1: hT = relu(Wi_slice.T @ X.T)  [ISL, CAP] ----
                  h_sb = hpool.tile([P, ICS, CAP], DT, tag="h", name="h_sb")
                  for i in range(ICS):
                      for n in range(NN):
                          pt = ps1pool.tile([P, NF], dt.float32, tag="ps1",
                                            name=f"ps1_{i}_{n}")
                          for k in range(KO):
                              nc.tensor.matmul(
                                  pt[:],
                                  wi_sb[:, k, i * P:(i + 1) * P],
                                  x_sb[:, k, n * NF:(n + 1) * NF],
                                  start=(k == 0),
                                  stop=(k == KO - 1),
                              )
                          nc.vector.tensor_scalar_max(
                              h_sb[:, i, n * NF:(n + 1) * NF], pt[:], 0.0)

                  # ---- GEMM2: outT_part = Wo_slice.T @ hT  [H, CAP] ----
                  out_sb = opool.tile([P, KO, CAP], DT, tag="o", name="out_sb")
                  for m in range(KO):
                      for n in range(NN):
                          pt = ps2pool.tile([P, NF], dt.float32, tag="ps2",
                                            name=f"ps2_{m}_{n}")
                          for k in range(ICS):
                              nc.tensor.matmul(
                                  pt[:],
                                  wo_sb[:, k, m * P:(m + 1) * P],
                                  h_sb[:, k, n * NF:(n + 1) * NF],
                                  start=(k == 0),
                                  stop=(k == ICS - 1),
                              )
                          nc.vector.tensor_copy(
                              out_sb[:, m, n * NF:(n + 1) * NF], pt[:])
                  nc.scalar.dma_start(
                      o_r[:, :, e * CAP:(e + 1) * CAP], out_sb[:])
    nc.compile()
    return nc


def _get_nc(mm_dtype: str):
    if mm_dtype not in _CACHE:
        _CACHE[mm_dtype] = _build_nc(mm_dtype)
    return _CACHE[mm_dtype]


def _routing(x_flat: np.ndarray, router_w: np.ndarray, router_b: np.ndarray):
    """Replicate the reference router bit-for-bit where possible (jax CPU),
    returning top-2 values/indices [N_TOK, 2] (fp32/int)."""
    try:
        import jax
        import jax.numpy as jnp

        cpu = jax.devices("cpu")[0]
        with jax.default_device(cpu):
            xj = jnp.asarray(x_flat.reshape(B, S, H))
            logits = jnp.einsum("bsh,eh->bse", xj, jnp.asarray(router_w)) \
                + jnp.asarray(router_b)
            wflat = jax.nn.softmax(logits, axis=-1).reshape(N_TOK, E)
            topv, topi = jax.lax.top_k(wflat, TOP_K)
            return np.asarray(topv), np.asarray(topi)
    except Exception:
        # numpy fallback (float64 logits for a stable ordering)
        logits = x_flat.astype(np.float64) @ router_w.astype(np.float64).T \
            + router_b.astype(np.float64)
        m = logits.max(axis=1, keepdims=True)
        ex = np.exp(logits - m)
        wflat = (ex / ex.sum(axis=1, keepdims=True)).astype(np.float32)
        topi = np.argsort(-wflat, axis=1, kind="stable")[:, :TOP_K]
        topv = np.take_along_axis(wflat, topi, axis=1)
        return topv, topi


def _dispatch(x_flat, topv, topi):
    """Capacity-limited dispatch: return disp_T [H, ER*CAP] (fp32) and the
    per-token slot bookkeeping used by the combine."""
    mask = np.zeros((N_TOK, E), dtype=bool)
    rows = np.arange(N_TOK)
    mask[rows[:, None], topi] = True
    expert_mask = mask[:, :ER]                       # [N, 7]
    pos = np.cumsum(expert_mask, axis=0, dtype=np.int32) - 1

    disp_T = np.zeros((H, ER * CAP), dtype=np.float32)
    for e in range(ER):
        idx_e = np.nonzero(expert_mask[:, e])[0][:CAP]
        disp_T[:, e * CAP:e * CAP + len(idx_e)] = x_flat[idx_e].T
    return disp_T, pos, rows


def make_in_maps(disp_T, experts_inter, experts_out):
    """Per-core input tensors for the SPMD program (bf16)."""
    import ml_dtypes

    bf = lambda a: np.ascontiguousarray(a.astype(ml_dtypes.bfloat16))
    xT_b = bf(disp_T)
    in_maps = []
    for c in range(N_CORES):
        sl = slice(c * ISL, (c + 1) * ISL)
        wi_c = experts_inter[:, :, sl].reshape(ER * H, ISL)
        wo_c = experts_out[:, sl, :].reshape(ER * ISL, H)
        in_maps.append({"xT": xT_b, "wi": bf(wi_c), "wo": bf(wo_c)})
    return in_maps


def kernel(x, router_w, router_b, experts_inter, experts_out):
    from concourse.bass_utils import run_bass_kernel_spmd

    x = np.ascontiguousarray(np.asarray(x, dtype=np.float32))
    router_w = np.asarray(router_w, dtype=np.float32)
    router_b = np.asarray(router_b, dtype=np.float32)
    experts_inter = np.asarray(experts_inter, dtype=np.float32)
    experts_out = np.asarray(experts_out, dtype=np.float32)

    x_flat = x.reshape(N_TOK, H)
    topv, topi = _routing(x_flat, router_w, router_b)
    disp_T, pos, rows = _dispatch(x_flat, topv, topi)

    in_maps = make_in_maps(disp_T, experts_inter, experts_out)

    nc = _get_nc(MM_DTYPE)
    res = run_bass_kernel_spmd(nc, in_maps, list(range(N_CORES)))
    global LAST_RESULT
    LAST_RESULT = res

    # sum bf16 partials in fp32: out_T [H, ER*CAP]
    out_T = np.zeros((H, ER * CAP), dtype=np.float32)
    for c in range(N_CORES):
        out_T += res.results[c]["outT"].astype(np.float32)

    # ---- host combine: pure gathers ----
    out_flat = np.ascontiguousarray(out_T.T)         # [ER*CAP, H]
    out_ext = np.vstack([out_flat, np.zeros((1, H), dtype=np.float32)])

    combined = np.zeros_like(x_flat)
    noop_w = np.zeros(N_TOK, dtype=np.float32)
    for k in range(TOP_K):
        e_k = topi[:, k]
        v_k = topv[:, k]
        is_noop = e_k == ER
        noop_w += np.where(is_noop, v_k, 0.0).astype(np.float32)
        p_k = pos[rows, np.minimum(e_k, ER - 1)]
        ok = (~is_noop) & (p_k < CAP)
        slot = np.where(ok, np.minimum(e_k, ER - 1) * CAP + p_k, ER * CAP)
        combined += out_ext[slot] * np.where(ok, v_k, 0.0)[:, None]
    combined += x_flat * noop_w[:, None]

    return combined.reshape(B, S, H)
